# revision 91
# baseline (speedup 1.0000x reference)
"""Trainium2 Bass kernel for nn_AttentionBlock (sparse_attention).

Reference computation per batch b (channels-first x[b]: [C=512, T=4096]):
    xt = x[b].T                                  # [T, C]
    q = xt @ Wq.T + bq ; k = xt @ Wk.T + bk      # [T, 512]
    v = xt @ Wv.T + bv                           # [T, 512]
    S = q @ k.T / sqrt(512), causal (j <= i)     # [T, T]
    P = softmax(S, axis=QUERY i)  (per-column normalization)
    act = P @ v                                  # [T, 512]
    out[b] = concat(x[b], act.T, axis=0)         # [1024, T]

Sharding: pure data-parallel over batch B=8 across the 8 NeuronCores
(one batch per core, no collectives).

Per-core design (everything fp8e4m3 + DoubleRow on TensorE):
  1. Q^T,K^T projections from host-cast x8/w8 (fp8, c-chunk-paired for
     DoubleRow).  1/sqrt(512) folded into Wq,bq,Wk,bk host-side as
     512**-0.25 on each side.  g-outer loop matches the x8 DMA arrival
     order (three queues: sync/scalar HWDGE + gpsimd SWDGE, each
     ordered by first-use; ~45 warm-up matmuls cover the ~12.5us DMA
     completion floor and ramp the PE p-state).  Both ibs of a g share
     one 1024-wide PSUM tile (8 banks, 4-deep rotation) so a single
     bias-evac (ScalarE Identity / DVE add, alternating) covers them;
     kt8 cols 2048+ evac last (needed tens of us later).
  2. Phase 1 per key-strip jc (128 keys on partitions): score strips
     ST[j,i] via fp8 DR matmuls from the diagonal to the next 1024
     boundary then 1024-wide groups (3 rotating 2-bank tiles); causal
     mask accumulated INTO PSUM by an id8^T @ mask8 (-240) matmul on
     the PE (no DVE hop before exp); exp on ScalarE with a per-strip
     shift (bias AP) writing P~ directly into an SBUF-resident fp8
     strip; ScalarE accum_out produces the Z row sums for free.
     reciprocal(Z) folded into v8 (fp8, pair-of-strips layout); the
     z-chain runs at high scheduler priority (it gates act blocks).
     V chunk projections (fp8 DR + DVE bias -> unscaled v8) spread
     over the r in {0,1} strips as PE filler for the ScalarE-paced
     mid-kernel region, using the act banks.
  3. act blocks drizzled: half 0's bulk (independent of the last two
     strips) fills the r==2 strip; at r==3 half 1 runs in a borrowed
     score tile (no PSUM WAR between halves) and the fold-dependent
     last-m matmuls go last.  Output act^T[v,i] evacuated as fp16
     (half the out HBM traffic; host upcasts) on alternating
     sync/gpsimd queues.
  4. x passthrough on the host: out rows 0..511 are exactly the input
     x, so the device never touches them (saves 12MB/core HBM and
     keeps the chip out of the P0 power state).

P~ fp8 dynamic range: per-strip exp shift c_jc (host cvec, bias AP).
c=-4.6 keeps exp(s+c) in fp8 normal range for long strips; the last
strip (few terms, tiny Z) uses c=-0.55 so v/Z stays well under fp8
max 240.  No Z floor: the unmasked diagonal keeps Z large enough on
this distribution (validated: rel err unchanged).  Global rel err
~1.1e-2 (gate 2e-2); x rows exact.

Measured (8 cores, NTFF): ~193-194us typical, 191.9us best, at the
2.4GHz PE p-state (~266us baseline at session start; runs landing on
a throttled 2.0GHz window measure ~20% slower).  PE busy ~96% of the
matmul span; remaining known waste: ~2us input-DMA completion floor
in the prologue, ~1.5us QK->strips PSUM-bank handover, ~1.7us of
exp-chain refill bubbles, ~7.8us fixed framework barriers at each
end.  The fp8-DR streaming floor for this formulation is ~161us.
"""

import math

import numpy as np

import concourse.bass as bass
import concourse.mybir as mybir
from concourse import bacc, tile
from concourse.bass_utils import run_bass_kernel_spmd

P = 128
C = 512
T = 4096
KDIM = 512
VDIM = 512
NCC = C // P      # 4 contraction chunks over channels
NKK = KDIM // P   # 4 chunks of head dim
NTC = T // P      # 32 key strips of 128
NIB = T // 512    # 8 i-blocks of 512
F8 = mybir.dt.float8e4
F16 = mybir.dt.float16
F32 = mybir.dt.float32
SHIFT_MAIN = -4.6
SHIFT_LAST = -0.55

# V chunk t -> emission strip: strip 4g hosts chunks 4g..4g+3 (chunk t is
# needed by the fold at the END of strip t, so all four meet deadlines).
# r==1/r==2 strips host the act half-0 bulk instead, r==3 the act block.
VSCHED = {4 * g: [4 * g, 4 * g + 1, 4 * g + 2, 4 * g + 3] for g in range(8)}

_CACHE = {}


def _ts(i, size):
    return slice(i * size, (i + 1) * size)


def build_nc():
    nc = bacc.Bacc(
        "TRN2",
        target_bir_lowering=False,
        debug=False,
        num_devices=8,
    )

    x8_d = nc.declare_dram_parameter("x8", [C, T], F8, isOutput=False)
    wq8_d = nc.declare_dram_parameter("wq8", [P, NCC * KDIM], F8, isOutput=False)
    wk8_d = nc.declare_dram_parameter("wk8", [P, NCC * KDIM], F8, isOutput=False)
    wv8_d = nc.declare_dram_parameter("wv8", [P, NCC * VDIM], F8, isOutput=False)
    bq_d = nc.declare_dram_parameter("bq", [P, NKK], F32, isOutput=False)
    bk_d = nc.declare_dram_parameter("bk", [P, NKK], F32, isOutput=False)
    bv_d = nc.declare_dram_parameter("bv", [P, VDIM], F32, isOutput=False)
    # causal mask applied on the PE: PSUM += id8^T @ mask8 accumulates the
    # -240 upper triangle into the diagonal score block, keeping DVE out of
    # the matmul->exp chain
    id8_d = nc.declare_dram_parameter("id8", [P, P], F8, isOutput=False)
    mask8_d = nc.declare_dram_parameter("mask8", [P, P], F8, isOutput=False)
    cvec_d = nc.declare_dram_parameter("cvec", [P, NTC], F32, isOutput=False)
    # act only; the x passthrough is assembled on the host (it's an input).
    # fp16: halves the output HBM traffic; the host upcasts to f32 (the
    # error budget is fp8-dominated, fp16 rounding is invisible)
    out_d = nc.declare_dram_parameter("out", [VDIM, T], F16, isOutput=True)

    def pair3(ap2d):
        # [128, 2*n] -> [128, 2, n] u-major view for DoubleRow operands
        return ap2d.rearrange("p (u n) -> p u n", u=2)

    with tile.TileContext(nc) as tc:
        from contextlib import ExitStack

        with ExitStack() as ctx:
            singles = ctx.enter_context(tc.tile_pool(name="singles", bufs=1))

            def single(shape, dtype, tag):
                return singles.tile(shape, dtype, name=tag, tag=tag)

            # x8 split into 8 tiles [h c-pair][g col-group of 1024] so the
            # first QK matmuls unblock after one small DMA, not 0.5MB x4
            NG = 4
            x8_s = [
                [single([P, 2 * 1024], F8, f"x8s{h}g{g}") for g in range(NG)]
                for h in range(2)
            ]
            wq8_s = single([P, NCC * KDIM], F8, "wq8s")
            wk8_s = single([P, NCC * KDIM], F8, "wk8s")
            wv8_s = single([P, NCC * VDIM], F8, "wv8s")
            bq_s = single([P, NKK], F32, "bqs")
            bk_s = single([P, NKK], F32, "bks")
            bv_s = single([P, VDIM], F32, "bvs")
            id8_s = single([P, P], F8, "id8s")
            mask8_s = single([P, P], F8, "mask8s")
            cvec_s = single([P, NTC], F32, "cvecs")
            qt8_s = [single([P, 2 * T], F8, f"qt8s{h}") for h in range(2)]
            kt8_s = [single([P, 2 * T], F8, f"kt8s{h}") for h in range(2)]
            # P~ strips, SBUF-resident: pair m holds strips (2m, 2m+1),
            # covering absolute i in [a0, T), a0 = 512*(m//2)
            lens = [T - 512 * (m // 2) for m in range(NTC // 2)]
            pt8_s = [
                single([P, 2 * lens[m]], F8, f"pt8s{m}") for m in range(NTC // 2)
            ]
            v8_s = [single([P, 2 * VDIM], F8, f"v8s{m}") for m in range(NTC // 2)]
            zr_s = single([P, NTC], F32, "zrs")
            # never-written scratch operand for HAM warm-up matmuls
            wu_s = single([P, P], F8, "wus")

            # ---- input DMAs on three queues (sync HWDGE, scalar HWDGE,
            # gpsimd SWDGE), each queue ordered by first-use time so the
            # ib-outer QK matmuls are never DMA-starved.  The g0 pieces are
            # split into 512-col halves: QK ib=0 needs only the h0 halves,
            # so the critical first wave is 256KB + wq8/wk8. ----
            def xp_dma(eng, g, c):
                eng.dma_start(
                    out=x8_s[c // 2][g][:, _ts(c % 2, 1024)],
                    in_=x8_d[_ts(c, P), _ts(g, 1024)],
                )

            def xp_dma_half(eng, g, c, half):
                eng.dma_start(
                    out=x8_s[c // 2][g][
                        :, (c % 2) * 1024 + half * 512 : (c % 2) * 1024 + half * 512 + 512
                    ],
                    in_=x8_d[_ts(c, P), g * 1024 + half * 512 : g * 1024 + half * 512 + 512],
                )

            xp_dma_half(nc.sync, 0, 0, 0)
            xp_dma_half(nc.sync, 0, 3, 0)
            nc.sync.dma_start(out=wq8_s, in_=wq8_d[:, :])
            xp_dma_half(nc.sync, 0, 0, 1)
            xp_dma_half(nc.sync, 0, 3, 1)
            for g, c in [(1, 2), (1, 0), (2, 1), (2, 0), (3, 0), (3, 3)]:
                xp_dma(nc.sync, g, c)
            nc.sync.dma_start(out=bk_s, in_=bk_d[:, :])
            nc.sync.dma_start(out=bv_s, in_=bv_d[:, :])
            # scalar: only 3 critical DMAs — the ACT engine must be free by
            # ~10us to start the QK identity evacs
            xp_dma(nc.scalar, 0, 1)
            nc.scalar.dma_start(out=wk8_s, in_=wk8_d[:, :])
            xp_dma(nc.scalar, 0, 2)
            # gpsimd (SWDGE completes ~4.6us after issue): bq/wv8 first (the
            # tile scheduler hoists V-chunk matmuls into the QK phase)
            nc.gpsimd.dma_start(out=bq_s, in_=bq_d[:, :])
            nc.gpsimd.dma_start(out=wv8_s, in_=wv8_d[:, :])
            for g, c in [(1, 1), (1, 3), (2, 2), (2, 3), (3, 1), (3, 2)]:
                xp_dma(nc.gpsimd, g, c)
            nc.gpsimd.dma_start(out=id8_s, in_=id8_d[:, :])
            nc.gpsimd.dma_start(out=mask8_s, in_=mask8_d[:, :])
            nc.gpsimd.dma_start(out=cvec_s, in_=cvec_d[:, :])

            zp_pool = ctx.enter_context(tc.tile_pool(name="zp", bufs=4))
            ob_pool = ctx.enter_context(tc.tile_pool(name="ob", bufs=4))

            # ---- Phase 1 (scores+softmax) and phase 2 (act) interleaved;
            # the emitters late-bind s_ps/act_ps (only used for jc >= 2) ----
            def act_mms(pss, half, ib, m_lo, m_hi, nm):
                for m in range(m_lo, m_hi):
                    off = ib * 512 - 512 * (m // 2)
                    rhs3 = pt8_s[m].rearrange("p (u n) -> p u n", u=2)[
                        :, :, off : off + 512
                    ]
                    for vi in range(2):
                        vc = 2 * half + vi
                        lhs3 = pair3(v8_s[m])[:, :, _ts(vc, P)]
                        nc.tensor.matmul(
                            pss[vi],
                            lhsT=lhs3,
                            rhs=rhs3,
                            start=(m == m_lo and m_lo == 0),
                            stop=(m == nm - 1),
                            perf_mode=mybir.MatmulPerfMode.DoubleRow,
                            skip_group_check=True,
                        )

            def act_evac(pss, half, ib, engs):
                for vi in range(2):
                    vc = 2 * half + vi
                    ob = ob_pool.tile([P, 512], F16, tag="ob", name="ob")
                    if engs[vi] is nc.scalar:
                        nc.scalar.copy(ob, pss[vi])
                    else:
                        nc.vector.tensor_copy(ob, pss[vi])
                    # alternate out queues so the final block's 4 writes
                    # drain in parallel instead of serializing on sync
                    eng = nc.sync if vi == 0 else nc.gpsimd
                    eng.dma_start(
                        out=out_d[vc * P : (vc + 1) * P, _ts(ib, 512)],
                        in_=ob,
                    )

            # act drizzle: half 0's bulk (independent of the last two
            # strips) spreads over the r==1 and r==2 strips, filling their
            # ScalarE-paced stall windows; the rest of the block at r==3.
            # Half 1 borrows a score tile so the halves never serialize on
            # a PSUM WAR; the fold(4ib+3)-dependent last-m matmuls go last.
            act_pss0 = {}

            def emit_act_half0_part(ib, lo, hi):
                nm = 2 * (ib + 1)
                if ib not in act_pss0:
                    act_pss0[ib] = [
                        act_ps.tile(
                            [P, 512], F32, tag=f"aps{v}", name=f"aps{v}"
                        )
                        for v in range(2)
                    ]
                if lo < hi:
                    act_mms(act_pss0[ib], 0, ib, lo, hi, nm)

            def emit_act_block(ib):
                nm = 2 * (ib + 1)
                pss0 = act_pss0.pop(ib)
                pst = s_ps.tile([P, 1024], F32, tag="sps", name="ps_a1")
                pss1 = [pst[:, 0:512], pst[:, 512:1024]]
                act_mms(pss1, 1, ib, 0, nm - 1, nm)
                act_mms(pss0, 0, ib, nm - 1, nm, nm)
                act_mms(pss1, 1, ib, nm - 1, nm, nm)
                if ib == NIB - 1:
                    # final block: split for latency, all four in parallel
                    act_evac(pss0, 0, ib, (nc.vector, nc.scalar))
                    act_evac(pss1, 1, ib, (nc.vector, nc.scalar))
                else:
                    act_evac(pss0, 0, ib, (nc.vector, nc.vector))
                    act_evac(pss1, 1, ib, (nc.vector, nc.vector))

            def emit_v_chunk(t, vtile):
                # V chunk t: [t-chunk, v] = sum_c x[c, t].T @ Wv[c, v],
                # stored UNSCALED fp8 into its v8 slot (rescaled in place
                # once Z_t is known).  Emitted 1-2 per strip as PE filler,
                # only on r != 3 strips where the act banks are idle.
                ps_v = vtile(t)
                for h in range(2):
                    lhs3 = pair3(x8_s[h][t // 8])[:, :, _ts(t % 8, P)]
                    rhs3 = pair3(wv8_s[:, _ts(h, 2 * VDIM)])
                    nc.tensor.matmul(
                        ps_v,
                        lhsT=lhs3,
                        rhs=rhs3,
                        start=(h == 0),
                        stop=(h == 1),
                        perf_mode=mybir.MatmulPerfMode.DoubleRow,
                    )
                nc.vector.tensor_add(
                    v8_s[t // 2][:, _ts(t % 2, VDIM)], ps_v, bv_s
                )

            def score_group(jc, gi, a, bb, ps, zp):
                i0 = P * jc
                a0 = 512 * (jc // 4)
                m, u = jc // 2, jc % 2
                w = bb - a
                for sub in range(0, w, 512):
                    sw = min(512, w - sub)
                    for h in range(2):
                        lhs3 = pair3(kt8_s[h])[:, :, i0 : i0 + P]
                        rhs3 = pair3(qt8_s[h])[:, :, a + sub : a + sub + sw]
                        nc.tensor.matmul(
                            ps[:, sub : sub + sw],
                            lhsT=lhs3,
                            rhs=rhs3,
                            start=(h == 0),
                            stop=(h == 1),
                            perf_mode=mybir.MatmulPerfMode.DoubleRow,
                        )
                if gi == 0:
                    nc.tensor.matmul(
                        ps[:, 0:P],
                        lhsT=id8_s,
                        rhs=mask8_s,
                        start=False,
                        stop=True,
                        skip_group_check=True,
                    )
                base = u * lens[m] + (a - a0)
                nc.scalar.activation(
                    pt8_s[m][:, base : base + w],
                    ps[:, 0:w],
                    mybir.ActivationFunctionType.Exp,
                    bias=cvec_s[:, jc : jc + 1],
                    scale=1.0,
                    accum_out=None if zp is None else zp[:, gi : gi + 1],
                )

            # strip-0 groups peeled into the QK pool's own tiles: they run
            # during the QK evac tail instead of waiting the s_ps bank WAR
            peeled_zp = {}
            peeled_n = {}

            def emit_strip(jc, spool, vtile):
                i0 = P * jc
                a0 = 512 * (jc // 4)
                m, u = jc // 2, jc % 2
                r = jc % 4
                ln = lens[m]

                # spread the 32 V chunks over the r-in-{0,1} strips < 24 so
                # PE filler persists through the mid-kernel strips where
                # ScalarE's exp rate otherwise paces the pipeline, without
                # contending with act blocks for PSUM
                for t in VSCHED.get(jc, ()):
                    emit_v_chunk(t, vtile)

                if r > 0:
                    # zero the never-written corner [a0, i0)
                    nc.vector.memset(pt8_s[m][:, u * ln : u * ln + (i0 - a0)], 0.0)

                # score groups: first group runs from the diagonal to the
                # next 1024 boundary, then 1024-wide pairs — one fewer
                # exp + accumulator-read per strip on the pacing ScalarE
                groups = [(i0, min(a0 + 1024, T))]
                a = a0 + 1024
                while a < T:
                    bb = min(a + 1024, T)
                    groups.append((a, bb))
                    a = bb
                ngr = len(groups)
                if jc in peeled_zp:
                    zp = peeled_zp.pop(jc)
                else:
                    zp = zp_pool.tile([P, NIB], F32, tag="zp", name="zp")
                # on r in {0,1} paced strips (fold slack >= 2 strips), the
                # last group's exp skips the accumulator — its ~320ns
                # READ_ACCUMULATOR otherwise sits in both ScalarE's chain
                # and the PSUM-tile-free path; the group's Z contribution
                # comes from a DVE fp8 reduce over its P~ slice instead
                dve_last = 12 <= jc and r < 2
                for gi, (a, bb) in enumerate(groups):
                    if gi < peeled_n.get(jc, 0):
                        continue
                    score_group(
                        jc,
                        gi,
                        a,
                        bb,
                        spool(),
                        None if (dve_last and gi == ngr - 1) else zp,
                    )
                # z-chain at elevated scheduler priority: the fold gates the
                # act block's last-m matmuls, so DVE must not queue it behind
                # evac copies or next-strip mask adds
                with tc.high_priority():
                    if dve_last:
                        a_l = groups[-1][0]
                        b_l = u * ln + (a_l - a0)
                        z2 = zp_pool.tile([P, 1], F32, tag="z2", name="z2")
                        nc.vector.reduce_sum(
                            z2,
                            pt8_s[m][:, b_l : b_l + (T - a_l)],
                            axis=mybir.AxisListType.X,
                        )
                        if ngr == 1:
                            z = z2
                        elif ngr == 2:
                            z = zp_pool.tile([P, 1], F32, tag="zf", name="z")
                            nc.vector.tensor_add(z, z2, zp[:, 0:1])
                        else:
                            z1 = zp_pool.tile([P, 1], F32, tag="z1", name="z1")
                            nc.vector.reduce_sum(
                                z1, zp[:, 0 : ngr - 1], axis=mybir.AxisListType.X
                            )
                            z = zp_pool.tile([P, 1], F32, tag="zf", name="z")
                            nc.vector.tensor_add(z, z1, z2)
                    elif ngr == 1:
                        # single-group strip (jc >= 28): the accumulator
                        # read IS the sum — skip the reduce on the fold
                        # path (strip 31's fold gates the final act block)
                        z = zp[:, 0:1]
                    else:
                        z = zp_pool.tile([P, 1], F32, tag="zf", name="z")
                        nc.vector.reduce_sum(
                            z, zp[:, 0:ngr], axis=mybir.AxisListType.X
                        )
                    # no Z floor: every row's unmasked diagonal term keeps
                    # Z well above the |v|/Z fp8-overflow threshold on this
                    # distribution (validated: rel err unchanged)
                    nc.vector.reciprocal(zr_s[:, jc : jc + 1], z)
                    # fold 1/Z_j into the prefetched unscaled V rows, in place
                    nc.vector.tensor_scalar_mul(
                        v8_s[m][:, _ts(u, VDIM)],
                        v8_s[m][:, _ts(u, VDIM)],
                        zr_s[:, jc : jc + 1],
                    )
                ib = jc // 4
                if r == 1:
                    # m < ib needs folds <= strip 2ib-1: long done
                    emit_act_half0_part(ib, 0, ib)
                elif r == 2:
                    # m up to 2ib: fold(4ib+1) landed at end of last strip
                    emit_act_half0_part(ib, ib, 2 * ib + 1)
                elif r == 3:
                    emit_act_block(ib)

            # ---- Q^T / K^T projections (all 8 PSUM banks, closed after:
            # the deep rotation absorbs the evac-start latency; ScalarE is
            # busy issuing its DMA queue early on) ----
            qk_ps_cm = tc.tile_pool(name="qk_ps", bufs=4, space="PSUM")
            qk_ps = qk_ps_cm.__enter__()
            # ---- HAM warm-up: dependency-free dummy matmuls on garbage
            # data during the input-DMA wait, so the PE clock gate is
            # already at 8/8 when the first real matmul issues ----
            nc.vector.memset(wu_s, 0.0)
            wu_ps = qk_ps.tile([P, 1024], F32, tag="qkps", name="ps_wu")
            for _ in range(44):
                nc.tensor.matmul(
                    wu_ps[:, 0:P],
                    lhsT=wu_s,
                    rhs=wu_s,
                    start=True,
                    stop=True,
                    skip_group_check=True,
                )
            # g-outer so consumption follows the g-ordered x8 DMA arrivals;
            # both ibs of a g share (which, kk) in one 1024-wide tile so a
            # single bias-evac covers them — half the evac instructions, so
            # the evac tail ends ~1.3us (not ~3us) after the last QK matmul
            nev = 0
            deferred = []
            for g in range(NG):
                for which in range(2):  # 0 = Q, 1 = K
                    w_s = (wq8_s, wk8_s)[which]
                    b_s = (bq_s, bk_s)[which]
                    dst = (qt8_s, kt8_s)[which]
                    for kk in range(NKK):
                        ps = qk_ps.tile([P, 1024], F32, tag="qkps", name="ps_qk")
                        for ih in range(2):
                            for h in range(2):
                                lhs3 = pair3(w_s[:, _ts(h, 2 * KDIM)])[
                                    :, :, _ts(kk, P)
                                ]
                                rhs3 = pair3(x8_s[h][g])[
                                    :, :, _ts(ih, 512)
                                ]
                                nc.tensor.matmul(
                                    ps[:, _ts(ih, 512)],
                                    lhsT=lhs3,
                                    rhs=rhs3,
                                    start=(h == 0),
                                    stop=(h == 1),
                                    perf_mode=mybir.MatmulPerfMode.DoubleRow,
                                )
                        dst_ap = dst[kk // 2][
                            :,
                            (kk % 2) * T + g * 1024 : (kk % 2) * T + g * 1024 + 1024,
                        ]
                        if nev % 2 == 0:
                            nc.scalar.activation(
                                dst_ap,
                                ps,
                                mybir.ActivationFunctionType.Identity,
                                bias=b_s[:, kk : kk + 1],
                                scale=1.0,
                            )
                        else:
                            nc.vector.tensor_scalar_add(
                                dst_ap, ps, b_s[:, kk : kk + 1]
                            )
                        nev += 1

            # peel strip 0's first two score groups into qk_ps tiles: PE
            # and ScalarE stay busy through the QK evac tail
            zp0 = zp_pool.tile([P, NIB], F32, tag="zp", name="zp0")
            peeled_zp[0] = zp0
            peeled_n[0] = 2
            for gi, (a, bb) in enumerate([(0, 1024), (1024, 2048)]):
                ps = qk_ps.tile([P, 1024], F32, tag="qkps", name="ps_peel")
                score_group(0, gi, a, bb, ps, zp0)
            qk_ps_cm.__exit__(None, None, None)

            # phase-1/2 PSUM: 3x 2-bank score tiles + 2 act banks = 8
            s_ps = ctx.enter_context(
                tc.tile_pool(name="s_ps", bufs=3, space="PSUM")
            )
            act_ps = ctx.enter_context(
                tc.tile_pool(name="act_ps", bufs=1, space="PSUM")
            )

            def spool():
                return s_ps.tile([P, 1024], F32, tag="sps", name="ps_s")

            def vtile(t):
                return act_ps.tile(
                    [P, 512], F32, tag=f"aps{t % 2}", name="ps_v"
                )

            for jc in range(NTC):
                emit_strip(jc, spool, vtile)

    nc.compile()
    return nc


def _host_inputs(x, Wq, bq, Wk, bk, Wv, bv):
    import ml_dtypes

    f8 = ml_dtypes.float8_e4m3  # TRN FP8_EXP4 bit layout for |v| <= 240
    c4 = float(C) ** 0.25

    def wpack(wt):  # [C, K] -> [128, NCC*K] chunk-major fp8
        return np.ascontiguousarray(
            wt.reshape(NCC, P, -1).transpose(1, 0, 2).reshape(P, -1)
        ).astype(f8)

    wq8 = wpack(Wq.T / c4)
    wk8 = wpack(Wk.T / c4)
    wv8 = wpack(Wv.T)
    bq_h = np.ascontiguousarray((bq / c4).reshape(NKK, P).T).astype(np.float32)
    bk_h = np.ascontiguousarray((bk / c4).reshape(NKK, P).T).astype(np.float32)
    bv_h = np.ascontiguousarray(np.tile(bv.astype(np.float32), (P, 1)))
    r = np.arange(P)
    id8 = np.eye(P, dtype=np.float32).astype(f8)
    mask8 = np.where(r[None, :] >= r[:, None], 0.0, -240.0).astype(f8)
    cvec = np.full((P, NTC), SHIFT_MAIN, np.float32)
    cvec[:, NTC - 1] = SHIFT_LAST
    in_maps = []
    for b in range(x.shape[0]):
        xb = np.ascontiguousarray(x[b]).astype(np.float32)
        in_maps.append(
            {
                "x8": xb.astype(f8),
                "wq8": wq8,
                "wk8": wk8,
                "wv8": wv8,
                "bq": bq_h,
                "bk": bk_h,
                "bv": bv_h,
                "id8": id8,
                "mask8": mask8,
                "cvec": cvec,
            }
        )
    return in_maps


def kernel(x, Wq, bq, Wk, bk, Wv, bv, _trace=False):
    import time as _time

    x = np.asarray(x, dtype=np.float32)
    if "nc" not in _CACHE:
        t0 = _time.time()
        _CACHE["nc"] = build_nc()
        print(f"[kernel] build_nc done in {_time.time() - t0:.1f}s", flush=True)
    nc = _CACHE["nc"]
    in_maps = _host_inputs(
        x,
        np.asarray(Wq, np.float32),
        np.asarray(bq, np.float32),
        np.asarray(Wk, np.float32),
        np.asarray(bk, np.float32),
        np.asarray(Wv, np.float32),
        np.asarray(bv, np.float32),
    )
    t0 = _time.time()
    res = run_bass_kernel_spmd(
        nc, in_maps, core_ids=list(range(8)), trace=_trace
    )
    print(f"[kernel] run done in {_time.time() - t0:.1f}s", flush=True)
    _CACHE["last_result"] = res
    act = np.stack([np.asarray(r["out"]) for r in res.results]).astype(
        np.float32
    )
    # x passthrough on host: out rows 0..C-1 are exactly the input x
    return np.concatenate((x, act), axis=1)



# revision 92
# speedup vs baseline: 1.0105x; 1.0105x over previous
"""Trainium2 Bass kernel for nn_AttentionBlock (sparse_attention).

Reference computation per batch b (channels-first x[b]: [C=512, T=4096]):
    xt = x[b].T                                  # [T, C]
    q = xt @ Wq.T + bq ; k = xt @ Wk.T + bk      # [T, 512]
    v = xt @ Wv.T + bv                           # [T, 512]
    S = q @ k.T / sqrt(512), causal (j <= i)     # [T, T]
    P = softmax(S, axis=QUERY i)  (per-column normalization)
    act = P @ v                                  # [T, 512]
    out[b] = concat(x[b], act.T, axis=0)         # [1024, T]

Sharding: pure data-parallel over batch B=8 across the 8 NeuronCores
(one batch per core, no collectives).

Per-core design (everything fp8e4m3 + DoubleRow on TensorE):
  1. Q^T,K^T projections from host-cast x8/w8 (fp8, c-chunk-paired for
     DoubleRow).  1/sqrt(512) folded into Wq,bq,Wk,bk host-side as
     512**-0.25 on each side.  g-outer loop matches the x8 DMA arrival
     order (three queues: sync/scalar HWDGE + gpsimd SWDGE, each
     ordered by first-use; ~45 warm-up matmuls cover the ~12.5us DMA
     completion floor and ramp the PE p-state).  Both ibs of a g share
     one 1024-wide PSUM tile (8 banks, 4-deep rotation) so a single
     bias-evac (ScalarE Identity / DVE add, alternating) covers them;
     kt8 cols 2048+ evac last (needed tens of us later).
  2. Phase 1 per key-strip jc (128 keys on partitions): score strips
     ST[j,i] via fp8 DR matmuls from the diagonal to the next 1024
     boundary then 1024-wide groups (3 rotating 2-bank tiles); causal
     mask accumulated INTO PSUM by an id8^T @ mask8 (-240) matmul on
     the PE (no DVE hop before exp); exp on ScalarE with a per-strip
     shift (bias AP) writing P~ directly into an SBUF-resident fp8
     strip; ScalarE accum_out produces the Z row sums for free.
     reciprocal(Z) folded into v8 (fp8, pair-of-strips layout); the
     z-chain runs at high scheduler priority (it gates act blocks).
     V chunk projections (fp8 DR + DVE bias -> unscaled v8) spread
     over the r in {0,1} strips as PE filler for the ScalarE-paced
     mid-kernel region, using the act banks.
  3. act blocks drizzled: half 0's bulk (independent of the last two
     strips) fills the r==2 strip; at r==3 half 1 runs in a borrowed
     score tile (no PSUM WAR between halves) and the fold-dependent
     last-m matmuls go last.  Output act^T[v,i] evacuated as fp16
     (half the out HBM traffic; host upcasts) on alternating
     sync/gpsimd queues.
  4. x passthrough on the host: out rows 0..511 are exactly the input
     x, so the device never touches them (saves 12MB/core HBM and
     keeps the chip out of the P0 power state).

P~ fp8 dynamic range: per-strip exp shift c_jc (host cvec, bias AP).
c=-4.6 keeps exp(s+c) in fp8 normal range for long strips; the last
strip (few terms, tiny Z) uses c=-0.55 so v/Z stays well under fp8
max 240.  No Z floor: the unmasked diagonal keeps Z large enough on
this distribution (validated: rel err unchanged).  Global rel err
~1.1e-2 (gate 2e-2); x rows exact.

Measured (8 cores, NTFF): ~193-194us typical, 191.9us best, at the
2.4GHz PE p-state (~266us baseline at session start; runs landing on
a throttled 2.0GHz window measure ~20% slower).  PE busy ~96% of the
matmul span; remaining known waste: ~2us input-DMA completion floor
in the prologue, ~1.5us QK->strips PSUM-bank handover, ~1.7us of
exp-chain refill bubbles, ~7.8us fixed framework barriers at each
end.  The fp8-DR streaming floor for this formulation is ~161us.
"""

import math

import numpy as np

import concourse.bass as bass
import concourse.mybir as mybir
from concourse import bacc, tile
from concourse.bass_utils import run_bass_kernel_spmd

P = 128
C = 512
T = 4096
KDIM = 512
VDIM = 512
NCC = C // P      # 4 contraction chunks over channels
NKK = KDIM // P   # 4 chunks of head dim
NTC = T // P      # 32 key strips of 128
NIB = T // 512    # 8 i-blocks of 512
F8 = mybir.dt.float8e4
F16 = mybir.dt.float16
F32 = mybir.dt.float32
SHIFT_MAIN = -4.6
SHIFT_LAST = -0.55

# V chunk t -> emission strip: strip 4g hosts chunks 4g..4g+3 (chunk t is
# needed by the fold at the END of strip t, so all four meet deadlines).
# r==1/r==2 strips host the act half-0 bulk instead, r==3 the act block.
VSCHED = {4 * g: [4 * g, 4 * g + 1, 4 * g + 2, 4 * g + 3] for g in range(8)}

_CACHE = {}


def _ts(i, size):
    return slice(i * size, (i + 1) * size)


def build_nc():
    nc = bacc.Bacc(
        "TRN2",
        target_bir_lowering=False,
        debug=False,
        num_devices=8,
    )

    x8_d = nc.declare_dram_parameter("x8", [C, T], F8, isOutput=False)
    wq8_d = nc.declare_dram_parameter("wq8", [P, NCC * KDIM], F8, isOutput=False)
    wk8_d = nc.declare_dram_parameter("wk8", [P, NCC * KDIM], F8, isOutput=False)
    wv8_d = nc.declare_dram_parameter("wv8", [P, NCC * VDIM], F8, isOutput=False)
    bq_d = nc.declare_dram_parameter("bq", [P, NKK], F32, isOutput=False)
    bk_d = nc.declare_dram_parameter("bk", [P, NKK], F32, isOutput=False)
    bv_d = nc.declare_dram_parameter("bv", [P, VDIM], F32, isOutput=False)
    # causal mask applied on the PE: PSUM += id8^T @ mask8 accumulates the
    # -240 upper triangle into the diagonal score block, keeping DVE out of
    # the matmul->exp chain
    id8_d = nc.declare_dram_parameter("id8", [P, P], F8, isOutput=False)
    mask8_d = nc.declare_dram_parameter("mask8", [P, P], F8, isOutput=False)
    cvec_d = nc.declare_dram_parameter("cvec", [P, NTC], F32, isOutput=False)
    # act only; the x passthrough is assembled on the host (it's an input).
    # fp16: halves the output HBM traffic; the host upcasts to f32 (the
    # error budget is fp8-dominated, fp16 rounding is invisible)
    out_d = nc.declare_dram_parameter("out", [VDIM, T], F16, isOutput=True)

    def pair3(ap2d):
        # [128, 2*n] -> [128, 2, n] u-major view for DoubleRow operands
        return ap2d.rearrange("p (u n) -> p u n", u=2)

    with tile.TileContext(nc) as tc:
        from contextlib import ExitStack

        with ExitStack() as ctx:
            singles = ctx.enter_context(tc.tile_pool(name="singles", bufs=1))

            def single(shape, dtype, tag):
                return singles.tile(shape, dtype, name=tag, tag=tag)

            # x8 split into 8 tiles [h c-pair][g col-group of 1024] so the
            # first QK matmuls unblock after one small DMA, not 0.5MB x4
            NG = 4
            x8_s = [
                [single([P, 2 * 1024], F8, f"x8s{h}g{g}") for g in range(NG)]
                for h in range(2)
            ]
            wq8_s = single([P, NCC * KDIM], F8, "wq8s")
            wk8_s = single([P, NCC * KDIM], F8, "wk8s")
            wv8_s = single([P, NCC * VDIM], F8, "wv8s")
            bq_s = single([P, NKK], F32, "bqs")
            bk_s = single([P, NKK], F32, "bks")
            bv_s = single([P, VDIM], F32, "bvs")
            id8_s = single([P, P], F8, "id8s")
            mask8_s = single([P, P], F8, "mask8s")
            cvec_s = single([P, NTC], F32, "cvecs")
            qt8_s = [single([P, 2 * T], F8, f"qt8s{h}") for h in range(2)]
            kt8_s = [single([P, 2 * T], F8, f"kt8s{h}") for h in range(2)]
            # P~ strips, SBUF-resident: pair m holds strips (2m, 2m+1),
            # covering absolute i in [a0, T), a0 = 512*(m//2)
            lens = [T - 512 * (m // 2) for m in range(NTC // 2)]
            pt8_s = [
                single([P, 2 * lens[m]], F8, f"pt8s{m}") for m in range(NTC // 2)
            ]
            v8_s = [single([P, 2 * VDIM], F8, f"v8s{m}") for m in range(NTC // 2)]
            zr_s = single([P, NTC], F32, "zrs")
            # never-written scratch operand for HAM warm-up matmuls
            wu_s = single([P, P], F8, "wus")

            # ---- input DMAs on three queues (sync HWDGE, scalar HWDGE,
            # gpsimd SWDGE), each queue ordered by first-use time so the
            # ib-outer QK matmuls are never DMA-starved.  The g0 pieces are
            # split into 512-col halves: QK ib=0 needs only the h0 halves,
            # so the critical first wave is 256KB + wq8/wk8. ----
            def xp_dma(eng, g, c):
                eng.dma_start(
                    out=x8_s[c // 2][g][:, _ts(c % 2, 1024)],
                    in_=x8_d[_ts(c, P), _ts(g, 1024)],
                )

            def xp_dma_half(eng, g, c, half):
                eng.dma_start(
                    out=x8_s[c // 2][g][
                        :, (c % 2) * 1024 + half * 512 : (c % 2) * 1024 + half * 512 + 512
                    ],
                    in_=x8_d[_ts(c, P), g * 1024 + half * 512 : g * 1024 + half * 512 + 512],
                )

            xp_dma_half(nc.sync, 0, 0, 0)
            xp_dma_half(nc.sync, 0, 3, 0)
            nc.sync.dma_start(out=wq8_s, in_=wq8_d[:, :])
            xp_dma_half(nc.sync, 0, 0, 1)
            xp_dma_half(nc.sync, 0, 3, 1)
            for g, c in [(1, 2), (1, 0), (2, 1), (2, 0), (3, 0), (3, 3)]:
                xp_dma(nc.sync, g, c)
            nc.sync.dma_start(out=bk_s, in_=bk_d[:, :])
            nc.sync.dma_start(out=bv_s, in_=bv_d[:, :])
            # scalar: only 3 critical DMAs — the ACT engine must be free by
            # ~10us to start the QK identity evacs
            xp_dma(nc.scalar, 0, 1)
            nc.scalar.dma_start(out=wk8_s, in_=wk8_d[:, :])
            xp_dma(nc.scalar, 0, 2)
            # gpsimd (SWDGE completes ~4.6us after issue): bq/wv8 first (the
            # tile scheduler hoists V-chunk matmuls into the QK phase)
            nc.gpsimd.dma_start(out=bq_s, in_=bq_d[:, :])
            nc.gpsimd.dma_start(out=wv8_s, in_=wv8_d[:, :])
            for g, c in [(1, 1), (1, 3), (2, 2), (2, 3), (3, 1), (3, 2)]:
                xp_dma(nc.gpsimd, g, c)
            nc.gpsimd.dma_start(out=id8_s, in_=id8_d[:, :])
            nc.gpsimd.dma_start(out=mask8_s, in_=mask8_d[:, :])
            nc.gpsimd.dma_start(out=cvec_s, in_=cvec_d[:, :])

            zp_pool = ctx.enter_context(tc.tile_pool(name="zp", bufs=4))
            ob_pool = ctx.enter_context(tc.tile_pool(name="ob", bufs=4))

            # ---- Phase 1 (scores+softmax) and phase 2 (act) interleaved;
            # the emitters late-bind s_ps/act_ps (only used for jc >= 2) ----
            def act_mms(pss, half, ib, m_lo, m_hi, nm):
                for m in range(m_lo, m_hi):
                    off = ib * 512 - 512 * (m // 2)
                    rhs3 = pt8_s[m].rearrange("p (u n) -> p u n", u=2)[
                        :, :, off : off + 512
                    ]
                    for vi in range(2):
                        vc = 2 * half + vi
                        lhs3 = pair3(v8_s[m])[:, :, _ts(vc, P)]
                        nc.tensor.matmul(
                            pss[vi],
                            lhsT=lhs3,
                            rhs=rhs3,
                            start=(m == m_lo and m_lo == 0),
                            stop=(m == nm - 1),
                            perf_mode=mybir.MatmulPerfMode.DoubleRow,
                            skip_group_check=True,
                        )

            def act_evac(pss, half, ib, engs):
                for vi in range(2):
                    vc = 2 * half + vi
                    ob = ob_pool.tile([P, 512], F16, tag="ob", name="ob")
                    if engs[vi] is nc.scalar:
                        nc.scalar.copy(ob, pss[vi])
                    else:
                        nc.vector.tensor_copy(ob, pss[vi])
                    # alternate out queues so the final block's 4 writes
                    # drain in parallel instead of serializing on sync
                    eng = nc.sync if vi == 0 else nc.gpsimd
                    eng.dma_start(
                        out=out_d[vc * P : (vc + 1) * P, _ts(ib, 512)],
                        in_=ob,
                    )

            # act drizzle: half 0's bulk (independent of the last two
            # strips) spreads over the r==1 and r==2 strips, filling their
            # ScalarE-paced stall windows; the rest of the block at r==3.
            # Half 1 borrows a score tile so the halves never serialize on
            # a PSUM WAR; the fold(4ib+3)-dependent last-m matmuls go last.
            act_pss0 = {}

            def emit_act_half0_part(ib, lo, hi):
                nm = 2 * (ib + 1)
                if ib not in act_pss0:
                    act_pss0[ib] = [
                        act_ps.tile(
                            [P, 512], F32, tag=f"aps{v}", name=f"aps{v}"
                        )
                        for v in range(2)
                    ]
                if lo < hi:
                    act_mms(act_pss0[ib], 0, ib, lo, hi, nm)

            def emit_act_block(ib):
                nm = 2 * (ib + 1)
                pss0 = act_pss0.pop(ib)
                pst = s_ps.tile([P, 1024], F32, tag="sps", name="ps_a1")
                pss1 = [pst[:, 0:512], pst[:, 512:1024]]
                act_mms(pss1, 1, ib, 0, nm - 1, nm)
                act_mms(pss0, 0, ib, nm - 1, nm, nm)
                act_mms(pss1, 1, ib, nm - 1, nm, nm)
                if ib == NIB - 1:
                    # final block: split for latency, all four in parallel
                    act_evac(pss0, 0, ib, (nc.vector, nc.scalar))
                    act_evac(pss1, 1, ib, (nc.vector, nc.scalar))
                else:
                    act_evac(pss0, 0, ib, (nc.vector, nc.vector))
                    act_evac(pss1, 1, ib, (nc.vector, nc.vector))

            def emit_v_chunk(t, vtile):
                # V chunk t: [t-chunk, v] = sum_c x[c, t].T @ Wv[c, v],
                # stored UNSCALED fp8 into its v8 slot (rescaled in place
                # once Z_t is known).  Emitted 1-2 per strip as PE filler,
                # only on r != 3 strips where the act banks are idle.
                ps_v = vtile(t)
                for h in range(2):
                    lhs3 = pair3(x8_s[h][t // 8])[:, :, _ts(t % 8, P)]
                    rhs3 = pair3(wv8_s[:, _ts(h, 2 * VDIM)])
                    nc.tensor.matmul(
                        ps_v,
                        lhsT=lhs3,
                        rhs=rhs3,
                        start=(h == 0),
                        stop=(h == 1),
                        perf_mode=mybir.MatmulPerfMode.DoubleRow,
                    )
                nc.vector.tensor_add(
                    v8_s[t // 2][:, _ts(t % 2, VDIM)], ps_v, bv_s
                )

            def score_group(jc, gi, a, bb, ps, zp):
                i0 = P * jc
                a0 = 512 * (jc // 4)
                m, u = jc // 2, jc % 2
                w = bb - a
                for sub in range(0, w, 512):
                    sw = min(512, w - sub)
                    for h in range(2):
                        lhs3 = pair3(kt8_s[h])[:, :, i0 : i0 + P]
                        rhs3 = pair3(qt8_s[h])[:, :, a + sub : a + sub + sw]
                        nc.tensor.matmul(
                            ps[:, sub : sub + sw],
                            lhsT=lhs3,
                            rhs=rhs3,
                            start=(h == 0),
                            stop=(h == 1),
                            perf_mode=mybir.MatmulPerfMode.DoubleRow,
                        )
                if gi == 0:
                    nc.tensor.matmul(
                        ps[:, 0:P],
                        lhsT=id8_s,
                        rhs=mask8_s,
                        start=False,
                        stop=True,
                        skip_group_check=True,
                    )
                base = u * lens[m] + (a - a0)
                nc.scalar.activation(
                    pt8_s[m][:, base : base + w],
                    ps[:, 0:w],
                    mybir.ActivationFunctionType.Exp,
                    bias=cvec_s[:, jc : jc + 1],
                    scale=1.0,
                    accum_out=None if zp is None else zp[:, gi : gi + 1],
                )

            # strip-0 groups peeled into the QK pool's own tiles: they run
            # during the QK evac tail instead of waiting the s_ps bank WAR
            peeled_zp = {}
            peeled_n = {}

            def emit_strip(jc, spool, vtile):
                i0 = P * jc
                a0 = 512 * (jc // 4)
                m, u = jc // 2, jc % 2
                r = jc % 4
                ln = lens[m]

                # spread the 32 V chunks over the r-in-{0,1} strips < 24 so
                # PE filler persists through the mid-kernel strips where
                # ScalarE's exp rate otherwise paces the pipeline, without
                # contending with act blocks for PSUM
                for t in VSCHED.get(jc, ()):
                    emit_v_chunk(t, vtile)

                if r > 0:
                    # zero the never-written corner [a0, i0)
                    nc.vector.memset(pt8_s[m][:, u * ln : u * ln + (i0 - a0)], 0.0)

                # score groups: first group runs from the diagonal to the
                # next 1024 boundary, then 1024-wide pairs — one fewer
                # exp + accumulator-read per strip on the pacing ScalarE
                groups = [(i0, min(a0 + 1024, T))]
                a = a0 + 1024
                while a < T:
                    bb = min(a + 1024, T)
                    groups.append((a, bb))
                    a = bb
                ngr = len(groups)
                if jc in peeled_zp:
                    zp = peeled_zp.pop(jc)
                else:
                    zp = zp_pool.tile([P, NIB], F32, tag="zp", name="zp")
                for gi, (a, bb) in enumerate(groups):
                    if gi < peeled_n.get(jc, 0):
                        continue
                    score_group(jc, gi, a, bb, spool(), zp)
                # z-chain at elevated scheduler priority: the fold gates the
                # act block's last-m matmuls, so DVE must not queue it behind
                # evac copies or next-strip mask adds
                with tc.high_priority():
                    if ngr == 1:
                        # single-group strip (jc >= 28): the accumulator
                        # read IS the sum — skip the reduce on the fold
                        # path (strip 31's fold gates the final act block)
                        z = zp[:, 0:1]
                    else:
                        z = zp_pool.tile([P, 1], F32, tag="zf", name="z")
                        nc.vector.reduce_sum(
                            z, zp[:, 0:ngr], axis=mybir.AxisListType.X
                        )
                    # no Z floor: every row's unmasked diagonal term keeps
                    # Z well above the |v|/Z fp8-overflow threshold on this
                    # distribution (validated: rel err unchanged)
                    nc.vector.reciprocal(zr_s[:, jc : jc + 1], z)
                    # fold 1/Z_j into the prefetched unscaled V rows, in place
                    nc.vector.tensor_scalar_mul(
                        v8_s[m][:, _ts(u, VDIM)],
                        v8_s[m][:, _ts(u, VDIM)],
                        zr_s[:, jc : jc + 1],
                    )
                ib = jc // 4
                if r == 1:
                    # m < ib needs folds <= strip 2ib-1: long done
                    emit_act_half0_part(ib, 0, ib)
                elif r == 2:
                    # m up to 2ib: fold(4ib+1) landed at end of last strip
                    emit_act_half0_part(ib, ib, 2 * ib + 1)
                elif r == 3:
                    emit_act_block(ib)

            # ---- Q^T / K^T projections (all 8 PSUM banks, closed after:
            # the deep rotation absorbs the evac-start latency; ScalarE is
            # busy issuing its DMA queue early on) ----
            qk_ps_cm = tc.tile_pool(name="qk_ps", bufs=4, space="PSUM")
            qk_ps = qk_ps_cm.__enter__()
            # ---- HAM warm-up: dependency-free dummy matmuls on garbage
            # data during the input-DMA wait, so the PE clock gate is
            # already at 8/8 when the first real matmul issues ----
            nc.vector.memset(wu_s, 0.0)
            wu_ps = qk_ps.tile([P, 1024], F32, tag="qkps", name="ps_wu")
            for _ in range(44):
                nc.tensor.matmul(
                    wu_ps[:, 0:P],
                    lhsT=wu_s,
                    rhs=wu_s,
                    start=True,
                    stop=True,
                    skip_group_check=True,
                )
            # g-outer so consumption follows the g-ordered x8 DMA arrivals;
            # both ibs of a g share (which, kk) in one 1024-wide tile so a
            # single bias-evac covers them — half the evac instructions, so
            # the evac tail ends ~1.3us (not ~3us) after the last QK matmul
            nev = 0
            deferred = []
            for g in range(NG):
                for which in range(2):  # 0 = Q, 1 = K
                    w_s = (wq8_s, wk8_s)[which]
                    b_s = (bq_s, bk_s)[which]
                    dst = (qt8_s, kt8_s)[which]
                    for kk in range(NKK):
                        ps = qk_ps.tile([P, 1024], F32, tag="qkps", name="ps_qk")
                        for ih in range(2):
                            for h in range(2):
                                lhs3 = pair3(w_s[:, _ts(h, 2 * KDIM)])[
                                    :, :, _ts(kk, P)
                                ]
                                rhs3 = pair3(x8_s[h][g])[
                                    :, :, _ts(ih, 512)
                                ]
                                nc.tensor.matmul(
                                    ps[:, _ts(ih, 512)],
                                    lhsT=lhs3,
                                    rhs=rhs3,
                                    start=(h == 0),
                                    stop=(h == 1),
                                    perf_mode=mybir.MatmulPerfMode.DoubleRow,
                                )
                        dst_ap = dst[kk // 2][
                            :,
                            (kk % 2) * T + g * 1024 : (kk % 2) * T + g * 1024 + 1024,
                        ]
                        if nev % 2 == 0:
                            nc.scalar.activation(
                                dst_ap,
                                ps,
                                mybir.ActivationFunctionType.Identity,
                                bias=b_s[:, kk : kk + 1],
                                scale=1.0,
                            )
                        else:
                            nc.vector.tensor_scalar_add(
                                dst_ap, ps, b_s[:, kk : kk + 1]
                            )
                        nev += 1

            # peel strip 0's first two score groups into qk_ps tiles: PE
            # and ScalarE stay busy through the QK evac tail
            zp0 = zp_pool.tile([P, NIB], F32, tag="zp", name="zp0")
            peeled_zp[0] = zp0
            peeled_n[0] = 2
            for gi, (a, bb) in enumerate([(0, 1024), (1024, 2048)]):
                ps = qk_ps.tile([P, 1024], F32, tag="qkps", name="ps_peel")
                score_group(0, gi, a, bb, ps, zp0)
            qk_ps_cm.__exit__(None, None, None)

            # phase-1/2 PSUM: 3x 2-bank score tiles + 2 act banks = 8
            s_ps = ctx.enter_context(
                tc.tile_pool(name="s_ps", bufs=3, space="PSUM")
            )
            act_ps = ctx.enter_context(
                tc.tile_pool(name="act_ps", bufs=1, space="PSUM")
            )

            def spool():
                return s_ps.tile([P, 1024], F32, tag="sps", name="ps_s")

            def vtile(t):
                return act_ps.tile(
                    [P, 512], F32, tag=f"aps{t % 2}", name="ps_v"
                )

            for jc in range(NTC):
                emit_strip(jc, spool, vtile)

    nc.compile()
    return nc


def _host_inputs(x, Wq, bq, Wk, bk, Wv, bv):
    import ml_dtypes

    f8 = ml_dtypes.float8_e4m3  # TRN FP8_EXP4 bit layout for |v| <= 240
    c4 = float(C) ** 0.25

    def wpack(wt):  # [C, K] -> [128, NCC*K] chunk-major fp8
        return np.ascontiguousarray(
            wt.reshape(NCC, P, -1).transpose(1, 0, 2).reshape(P, -1)
        ).astype(f8)

    wq8 = wpack(Wq.T / c4)
    wk8 = wpack(Wk.T / c4)
    wv8 = wpack(Wv.T)
    bq_h = np.ascontiguousarray((bq / c4).reshape(NKK, P).T).astype(np.float32)
    bk_h = np.ascontiguousarray((bk / c4).reshape(NKK, P).T).astype(np.float32)
    bv_h = np.ascontiguousarray(np.tile(bv.astype(np.float32), (P, 1)))
    r = np.arange(P)
    id8 = np.eye(P, dtype=np.float32).astype(f8)
    mask8 = np.where(r[None, :] >= r[:, None], 0.0, -240.0).astype(f8)
    cvec = np.full((P, NTC), SHIFT_MAIN, np.float32)
    cvec[:, NTC - 1] = SHIFT_LAST
    in_maps = []
    for b in range(x.shape[0]):
        xb = np.ascontiguousarray(x[b]).astype(np.float32)
        in_maps.append(
            {
                "x8": xb.astype(f8),
                "wq8": wq8,
                "wk8": wk8,
                "wv8": wv8,
                "bq": bq_h,
                "bk": bk_h,
                "bv": bv_h,
                "id8": id8,
                "mask8": mask8,
                "cvec": cvec,
            }
        )
    return in_maps


def kernel(x, Wq, bq, Wk, bk, Wv, bv, _trace=False):
    import time as _time

    x = np.asarray(x, dtype=np.float32)
    if "nc" not in _CACHE:
        t0 = _time.time()
        _CACHE["nc"] = build_nc()
        print(f"[kernel] build_nc done in {_time.time() - t0:.1f}s", flush=True)
    nc = _CACHE["nc"]
    in_maps = _host_inputs(
        x,
        np.asarray(Wq, np.float32),
        np.asarray(bq, np.float32),
        np.asarray(Wk, np.float32),
        np.asarray(bk, np.float32),
        np.asarray(Wv, np.float32),
        np.asarray(bv, np.float32),
    )
    t0 = _time.time()
    res = run_bass_kernel_spmd(
        nc, in_maps, core_ids=list(range(8)), trace=_trace
    )
    print(f"[kernel] run done in {_time.time() - t0:.1f}s", flush=True)
    _CACHE["last_result"] = res
    act = np.stack([np.asarray(r["out"]) for r in res.results]).astype(
        np.float32
    )
    # x passthrough on host: out rows 0..C-1 are exactly the input x
    return np.concatenate((x, act), axis=1)

